# revision 3
# baseline (speedup 1.0000x reference)
"""CGCNNConv fused kernel v2 — minimal HBM traffic (the device is DMA-BW bound).

Design vs baseline:
- No DRAM tables at all. The only per-edge HBM traffic is ONE gather of the
  raw 256B h[dst] row (f16). dst rows are expanded to the gate/cand
  preactivation on-chip: transpose h_dst tile on PE, matmul with wdst.
- src-side contribution: S_win = h_win @ wsrc + bias computed once per window
  (128 src nodes), expanded per edge with a transposed one-hot matmul.
- Edges sharded by src ownership; per core, dst ids are rank-packed against
  the core's unique-dst table and sorted by (window, rank<25088, rank); the
  lo/hi split keeps gather indices within int16. Pad slots gather row 0 and
  are killed by their -512 one-hot label.
- Scatter-add via one-hot matmul per tile (as baseline), BN stats on-chip,
  1KB AllReduce, residual h recovered from hT by on-chip transpose.
"""

import numpy as np

N_NODES = 50000
N_EDGES = 800000
D = 128
DE = 10
NCORES = 8
NB = N_NODES // NCORES          # 6250 nodes per core
NW = 49                         # windows of 128 src nodes
NLO = 25088                     # lo/hi split of dst index space (196*128)
HROWS = 50176                   # padded h row count (392*128)
SROWS = NW * 128                # 6272 = padded local nodes
BN_EPS = 1e-5


def _wrap_idx(flat16):
    """dma_gather index layout: flat[k] -> partition k%16 (replicated x8), free k//16."""
    n = flat16.shape[0]
    arr = flat16.reshape(n // 16, 16).T          # [16, n/16]
    return np.tile(arr, (8, 1))                  # [128, n/16] int16


def plan_layout(src, dst):
    """Static per-window tile layout shared by all cores.

    Returns (LO_T, HI_T): lists of per-window lo/hi tile counts
    (max over cores, T_w = LO_T+HI_T forced even), plus per-core edge
    membership for packing.
    """
    per_core = []
    lo_need = np.zeros(NW, dtype=np.int64)
    hi_need = np.zeros(NW, dtype=np.int64)
    for k in range(NCORES):
        base = k * NB
        sel = np.where((src >= base) & (src < base + NB))[0]
        s_loc = src[sel] - base
        d = dst[sel]
        # dense per-core packing: gather by rank of dst among this core's
        # unique dsts -> ascending, gap-free gather addresses (row-buffer hits)
        uniq, inv = np.unique(d, return_inverse=True)
        assert uniq.shape[0] <= HROWS
        order = np.lexsort((inv, inv >= NLO, s_loc // 128))
        sel = sel[order]
        s_loc = s_loc[order]
        inv = inv[order]
        win = s_loc // 128
        bounds = np.searchsorted(win, np.arange(NW + 1))
        for w in range(NW):
            lo_, hi_ = bounds[w], bounds[w + 1]
            rw = inv[lo_:hi_]
            nlo = int((rw < NLO).sum())
            nhi = int(rw.shape[0] - nlo)
            lo_need[w] = max(lo_need[w], (nlo + 127) // 128)
            hi_need[w] = max(hi_need[w], (nhi + 127) // 128)
        per_core.append((sel, s_loc, inv, bounds, uniq))
    LO_T = lo_need.copy()
    HI_T = hi_need.copy()
    for w in range(NW):
        if (LO_T[w] + HI_T[w]) % 2 == 1:
            HI_T[w] += 1
    return [int(x) for x in LO_T], [int(x) for x in HI_T], per_core


def pack_core(core_data, LO_T, HI_T, ef_t):
    """Build one core's idx_all [128, CTOT] i32 and eft [11, 128*TT] f16."""
    sel, s_loc, inv, bounds, uniq = core_data
    TT = sum(LO_T) + sum(HI_T)
    eft = np.zeros((11, 128 * TT), dtype=np.float16)
    cols = []
    tile_off = 0
    for w in range(NW):
        lo_, hi_ = bounds[w], bounds[w + 1]
        dw = inv[lo_:hi_]
        sw = s_loc[lo_:hi_] - 128 * w
        ew = sel[lo_:hi_]
        is_lo = dw < NLO
        nlo = int(is_lo.sum())
        nhi = int(dw.shape[0] - nlo)
        LO_CAP = LO_T[w] * 128
        HI_CAP = HI_T[w] * 128
        T_w = LO_T[w] + HI_T[w]
        assert nlo <= LO_CAP and nhi <= HI_CAP

        lo_idx = np.zeros(LO_CAP, dtype=np.int16)
        lo_idx[:nlo] = (dw[is_lo] // 2).astype(np.int16)
        hi_idx = np.zeros(HI_CAP, dtype=np.int16)
        hi_idx[:nhi] = ((dw[~is_lo] - NLO) // 2).astype(np.int16)

        pr = np.zeros(T_w * 128, dtype=np.float16)
        pr[:nlo] = (dw[is_lo] & 1).astype(np.float16)
        pr[LO_CAP:LO_CAP + nhi] = (dw[~is_lo] & 1).astype(np.float16)

        wl = np.full(T_w * 128, -512.0, dtype=np.float16)
        wl[:nlo] = sw[is_lo].astype(np.float16)
        wl[LO_CAP:LO_CAP + nhi] = sw[~is_lo].astype(np.float16)

        base_col = 128 * tile_off
        eft[0:DE, base_col:base_col + 128 * T_w][:, :nlo] = ef_t[:, ew[is_lo]]
        eft[0:DE, base_col + LO_CAP:base_col + LO_CAP + nhi] = ef_t[:, ew[~is_lo]]
        eft[DE, base_col:base_col + 128 * T_w] = wl

        wl_p = wl.reshape(T_w, 128).T.astype(np.float16)   # [128, T_w]
        pr_p = pr.reshape(T_w, 128).T.astype(np.float16)
        if T_w % 2 == 1:
            wl_p = np.concatenate([wl_p, np.full((128, 1), -512.0, np.float16)], axis=1)
            pr_p = np.concatenate([pr_p, np.zeros((128, 1), np.float16)], axis=1)
        blk = np.concatenate([
            _wrap_idx(lo_idx).view(np.int32),
            _wrap_idx(hi_idx).view(np.int32),
            np.ascontiguousarray(wl_p).view(np.int32),
            np.ascontiguousarray(pr_p).view(np.int32),
        ], axis=1)
        cols.append(blk)
        tile_off += T_w
    idx_all = np.concatenate(cols, axis=1)
    return np.ascontiguousarray(idx_all), eft


def _setup_act_tables():
    """Single combined exp+ln act table set (avoids per-transition reloads)."""
    import os, json, glob, shutil, tempfile
    if os.environ.get("BASS_ACT_ROOT_JSON_PATH"):
        return
    import neuronxcc
    cand = glob.glob(os.path.join(os.path.dirname(neuronxcc.__file__),
                                  "pwp", "pwp_bin_*", "act_info.json"))
    srcj = None
    for c in cand:
        dd = json.load(open(c))
        names = [s.get("name") for s in dd.get("act_func_sets", [])]
        if "natural_log_exp_and_others" in names:
            srcj = c
            break
    if srcj is None:
        return
    dstdir = os.path.join(tempfile.gettempdir(), "act_nlexp_only")
    os.makedirs(dstdir, exist_ok=True)
    dd = json.load(open(srcj))
    keep = [s for s in dd["act_func_sets"] if s["name"] == "natural_log_exp_and_others"]
    dd["act_func_sets"] = keep
    srcdir = os.path.dirname(srcj)
    for s in keep:
        for key in ("bkt_bin", "ctrl_bin", "profile_json"):
            f = s.get(key)
            if f and not os.path.exists(os.path.join(dstdir, f)):
                shutil.copy(os.path.join(srcdir, f), os.path.join(dstdir, f))
    for f in glob.glob(os.path.join(srcdir, "*.bin")) + glob.glob(os.path.join(srcdir, "*.json")):
        b = os.path.basename(f)
        if b != "act_info.json" and not os.path.exists(os.path.join(dstdir, b)):
            try:
                os.symlink(f, os.path.join(dstdir, b))
            except OSError:
                pass
    with open(os.path.join(dstdir, "act_info.json"), "w") as fh:
        json.dump(dd, fh)
    os.environ["BASS_ACT_ROOT_JSON_PATH"] = os.path.join(dstdir, "act_info.json")

    import concourse.hw_specs as hw_specs
    import concourse.bacc as bacc_mod
    import concourse.mybir as mybir
    tables = {keep[0]["name"]: {mybir.ActivationFunctionType.from_pwp(v)
                                for v in keep[0]["act"].keys()}}

    def _patched(module_arch):
        return tables
    hw_specs.get_activation_tables = _patched
    bacc_mod.get_activation_tables = _patched


def _build_nc(LO_T, HI_T):
    import concourse.bass as bass
    import concourse.bacc as bacc
    import concourse.mybir as mybir
    import concourse.tile as tile
    from concourse.masks import make_identity

    f16, f32, i32, i16 = (mybir.dt.float16, mybir.dt.float32,
                          mybir.dt.int32, mybir.dt.int16)
    AF = mybir.ActivationFunctionType
    OP = mybir.AluOpType
    P = 128

    T_W = [LO_T[w] + HI_T[w] for w in range(NW)]
    TT = sum(T_W)
    TMAX = max(T_W)
    # idx_all column offsets per window (int32 cols)
    C_W = [LO_T[w] * 4 + HI_T[w] * 4 + 2 * ((T_W[w] + 1) // 2) for w in range(NW)]
    COFF = np.concatenate([[0], np.cumsum(C_W)]).astype(int)
    TOFF = np.concatenate([[0], np.cumsum(T_W)]).astype(int)
    CMAX = max(C_W)

    nc = bacc.Bacc("TRN2", target_bir_lowering=False, debug=False,
                   num_devices=NCORES, num_swdge_queues=2)

    h16r = nc.dram_tensor("h16r", [HROWS, D], f16, kind="ExternalInput")
    hTs = nc.dram_tensor("hTs", [P, SROWS], f16, kind="ExternalInput")
    wsrc = nc.dram_tensor("wsrc", [P, 256], f16, kind="ExternalInput")
    wdst = nc.dram_tensor("wdst", [P, 256], f16, kind="ExternalInput")
    wef = nc.dram_tensor("wef", [DE, 256], f16, kind="ExternalInput")
    bias = nc.dram_tensor("bias", [P, 256], f32, kind="ExternalInput")
    eft = nc.dram_tensor("eft", [DE + 1, 128 * TT], f16, kind="ExternalInput")
    idx_all = nc.dram_tensor("idx_all", [P, int(COFF[-1])], i32, kind="ExternalInput")
    bng = nc.dram_tensor("bng", [1, D], f32, kind="ExternalInput")
    bnb = nc.dram_tensor("bnb", [1, D], f32, kind="ExternalInput")
    out_d = nc.dram_tensor("out", [SROWS, D], f32, kind="ExternalOutput")

    with tile.TileContext(nc) as tc:
        with (
            tc.tile_pool(name="const", bufs=1) as cp,
            tc.tile_pool(name="win", bufs=2) as wp,       # per-window big tiles
            tc.tile_pool(name="sm", bufs=4) as sp,        # small per-tile tiles
            tc.tile_pool(name="psPP", bufs=2, space="PSUM") as ppp,   # preact groups
            tc.tile_pool(name="psTR", bufs=2, space="PSUM") as ptr,   # transposes
            tc.tile_pool(name="psS", bufs=1, space="PSUM") as psS,
            tc.tile_pool(name="psB", bufs=1, space="PSUM") as psB,
            tc.tile_pool(name="psW", bufs=2, space="PSUM") as psW,
            tc.tile_pool(name="dram", bufs=1, space="DRAM") as dp,
        ):
            # ---------- constants ----------
            ident = cp.tile([P, P], f16)
            make_identity(nc, ident[:])
            iota_i = cp.tile([P, P], i16)
            nc.gpsimd.iota(iota_i[:], pattern=[[1, P]], base=0, channel_multiplier=0)
            iota_f = cp.tile([P, P], f16)
            nc.vector.tensor_copy(iota_f[:], iota_i[:])
            iotap_i = cp.tile([P, 1], i16)
            nc.gpsimd.iota(iotap_i[:], pattern=[[1, 1]], base=0, channel_multiplier=1)
            iotap_f = cp.tile([P, 1], f32)
            nc.vector.tensor_copy(iotap_f[:], iotap_i[:])
            ones1 = cp.tile([1, P], f16)
            nc.vector.memset(ones1[:], 1.0)
            ones_c = cp.tile([P, 1], f32)
            nc.vector.memset(ones_c[:], 1.0)

            wsrc_s = cp.tile([P, 256], f16)
            nc.sync.dma_start(wsrc_s[:], wsrc[:])
            wdst_s = cp.tile([P, 256], f16)
            nc.sync.dma_start(wdst_s[:], wdst[:])
            wef_s = cp.tile([DE, 256], f16)
            nc.sync.dma_start(wef_s[:], wef[:])
            bias_s = cp.tile([P, 256], f32)
            nc.sync.dma_start(bias_s[:], bias[:])

            agg = cp.tile([P, NW, D], f32)
            rstat = cp.tile([P, 256], f32)
            nc.vector.memset(rstat[:], 0.0)

            # pre-clear both rotation buffers of the gather destination
            # (trimmed trailing pad indices leave slots unwritten; initial
            # SBUF garbage could be NaN and would poison 0*NaN in matmuls)
            for _ in range(2):
                hd0 = wp.tile([P, TMAX, 256], f16, tag="hdst")
                nc.vector.memset(hd0[:], 0.0)

            # ---------- main edge loop ----------
            for w in range(NW):
                lo_t, hi_t, t_w = LO_T[w], HI_T[w], T_W[w]
                lo_cap, hi_cap = lo_t * 128, hi_t * 128
                ng = t_w // 2

                ia = wp.tile([P, CMAX], i32, tag="ia")
                nc.sync.dma_start(ia[:, :C_W[w]], idx_all[:, int(COFF[w]):int(COFF[w + 1])])
                li = ia[:, 0:lo_t * 4]
                hi_ = ia[:, lo_t * 4:lo_t * 4 + hi_t * 4]
                wcols = (t_w + 1) // 2
                wl = ia[:, lo_t * 4 + hi_t * 4:lo_t * 4 + hi_t * 4 + wcols].bitcast(f16)
                par = ia[:, lo_t * 4 + hi_t * 4 + wcols:C_W[w]].bitcast(f16)

                efts = wp.tile([DE, TMAX * 128], f16, tag="efts")
                nc.sync.dma_start(efts[:, :t_w * 128],
                                  eft[0:DE, int(TOFF[w]) * 128:int(TOFF[w + 1]) * 128])
                wlr = wp.tile([1, TMAX * 128], f16, tag="wlr")
                nc.sync.dma_start(wlr[:, :t_w * 128],
                                  eft[DE:DE + 1, int(TOFF[w]) * 128:int(TOFF[w + 1]) * 128])

                # S_win = h_win @ wsrc + bias  (on-chip, f16)
                hw_t = sp.tile([P, P], f16, tag="hwt")
                nc.sync.dma_start(hw_t[:], hTs[:, w * P:(w + 1) * P])
                ps_s = psS.tile([P, 256], f32, tag="swin")
                nc.tensor.matmul(ps_s[:], lhsT=hw_t[:], rhs=wsrc_s[:], start=True, stop=True)
                s16 = sp.tile([P, 256], f16, tag="s16")
                nc.vector.tensor_tensor(s16[:], ps_s[:], bias_s[:], op=OP.add)

                # gather h pair-rows (512B: ranks 2j, 2j+1); parity-blended below
                hdst = wp.tile([P, TMAX, 256], f16, tag="hdst")
                nc.gpsimd.dma_gather(
                    hdst[:, 0:lo_t, :],
                    h16r[0:NLO, :].rearrange("(a b) c -> a (b c)", b=2),
                    li.bitcast(i16), lo_cap, lo_cap, 256,
                    single_packet=False, queue_num=0)
                nc.gpsimd.dma_gather(
                    hdst[:, lo_t:t_w, :],
                    h16r[NLO:HROWS, :].rearrange("(a b) c -> a (b c)", b=2),
                    hi_.bitcast(i16), hi_cap, hi_cap, 256,
                    single_packet=False, queue_num=1)
                hdiff = wp.tile([P, TMAX, D], f16, tag="hdiff")
                nc.vector.tensor_tensor(hdiff[:, :t_w, :], hdst[:, :t_w, D:256],
                                        hdst[:, :t_w, 0:D], op=OP.subtract)
                hsel = wp.tile([P, TMAX, D], f16, tag="hsel")
                for t in range(t_w):
                    nc.vector.tensor_tensor(hsel[:, t, :], hdiff[:, t, :],
                                            par[:, t:t + 1].to_broadcast([P, P]),
                                            op=OP.mult)
                    nc.vector.tensor_tensor(hsel[:, t, :], hsel[:, t, :],
                                            hdst[:, t, 0:D], op=OP.add)

                # transposed one-hot (node -> edge) for the S expansion:
                # bcast wl along partitions via K=1 matmul, compare to iota_p
                ohT = wp.tile([P, TMAX * 128], f16, tag="ohT")
                nchunk = (t_w * 128 + 511) // 512
                for c in range(nchunk):
                    c0 = c * 512
                    c1 = min(t_w * 128, c0 + 512)
                    ps_b = psB.tile([P, 512], f32, tag="bcast")
                    nc.tensor.matmul(ps_b[:, :c1 - c0], lhsT=ones1[:],
                                     rhs=wlr[:, c0:c1], start=True, stop=True)
                    nc.vector.tensor_tensor(ohT[:, c0:c1],
                                            iotap_f[:].to_broadcast([P, c1 - c0]),
                                            ps_b[:, :c1 - c0], op=OP.is_equal)

                # transpose gathered h rows: [edge, feat] -> [feat, edge]
                hdT = wp.tile([P, TMAX, D], f16, tag="hdT")
                for q in range((t_w + 3) // 4):
                    q0 = q * 4
                    qn = min(4, t_w - q0)
                    ps_t = ptr.tile([P, 4, D], f16, tag="tr")
                    for j in range(qn):
                        nc.tensor.transpose(ps_t[:, j, :], hsel[:, q0 + j, :], ident[:])
                    nc.scalar.copy(hdT[:, q0:q0 + qn, :], ps_t[:, :qn, :])

                # preact per 2-tile group, act path
                e16 = wp.tile([P, TMAX * 256], f16, tag="e16")
                for g in range(ng):
                    t0 = 2 * g
                    pp = ppp.tile([P, 2, 256], f32, tag="pp")
                    for j in range(2):
                        t = t0 + j
                        nc.tensor.matmul(pp[:, j, :], lhsT=efts[:, t * 128:(t + 1) * 128],
                                         rhs=wef_s[:], start=True, stop=False)
                        nc.tensor.matmul(pp[:, j, :], lhsT=hdT[:, t, :],
                                         rhs=wdst_s[:], start=False, stop=False)
                        nc.tensor.matmul(pp[:, j, :], lhsT=ohT[:, t * 128:(t + 1) * 128],
                                         rhs=s16[:], start=False, stop=True)
                    nc.scalar.activation(e16[:, g * 512:(g + 1) * 512], pp[:], AF.Exp)

                u16 = wp.tile([P, TMAX * 256], f16, tag="u16")
                nc.scalar.activation(u16[:, :ng * 512], e16[:, :ng * 512], AF.Ln, bias=1.0)

                m16 = wp.tile([P, TMAX, D], f16, tag="m16")
                for g in range(ng):
                    g16 = sp.tile([P, 2, D], f16, tag="g16")
                    nc.scalar.activation(
                        g16[:], u16[:, g * 512:g * 512 + 512].rearrange("a (b c) -> a b c", b=2)[:, :, 0:D],
                        AF.Exp, scale=-1.0)
                    nc.vector.tensor_tensor(
                        m16[:, 2 * g:2 * g + 2, :], g16[:],
                        u16[:, g * 512:g * 512 + 512].rearrange("a (b c) -> a b c", b=2)[:, :, D:256],
                        op=OP.mult)

                # scatter-add via one-hot matmuls
                pw = psW.tile([P, D], f32, tag="winps")
                for t in range(t_w):
                    oh = sp.tile([P, P], f16, tag="oh")
                    nc.vector.tensor_tensor(oh[:], iota_f[:],
                                            wl[:, t:t + 1].to_broadcast([P, P]),
                                            op=OP.is_equal)
                    nc.tensor.matmul(pw[:], lhsT=oh[:], rhs=m16[:, t, :],
                                     start=(t == 0), stop=(t == t_w - 1))

                nc.vector.tensor_copy(agg[:, w, :], pw[:])
                sq = sp.tile([P, D], f32, tag="sq")
                nc.vector.tensor_tensor(sq[:], agg[:, w, :], agg[:, w, :], op=OP.mult)
                nc.vector.tensor_tensor(rstat[:, 0:D], rstat[:, 0:D], agg[:, w, :], op=OP.add)
                nc.vector.tensor_tensor(rstat[:, D:256], rstat[:, D:256], sq[:], op=OP.add)

            # ---------- BN stats + output ----------
            pstat = psS.tile([1, 256], f32, tag="swin")
            nc.tensor.matmul(pstat[:], lhsT=ones_c[:], rhs=rstat[:], start=True, stop=True)
            stat_l = cp.tile([1, 256], f32)
            nc.vector.tensor_copy(stat_l[:], pstat[:])

            cc_in = dp.tile([1, 256], f32)
            cc_out = dp.tile([1, 256], f32)
            nc.gpsimd.dma_start(cc_in[:], stat_l[:])
            nc.gpsimd.collective_compute(
                "AllReduce", mybir.AluOpType.add,
                replica_groups=[list(range(NCORES))],
                ins=[cc_in.opt()], outs=[cc_out.opt()])
            stat_g = cp.tile([1, 256], f32)
            nc.sync.dma_start(stat_g[:], cc_out[:])

            bng_s = cp.tile([1, D], f32)
            nc.sync.dma_start(bng_s[:], bng[:])
            bnb_s = cp.tile([1, D], f32)
            nc.sync.dma_start(bnb_s[:], bnb[:])

            mean = cp.tile([1, D], f32)
            nc.vector.tensor_scalar_mul(mean[:], stat_g[:, 0:D], 1.0 / N_NODES)
            ex2 = cp.tile([1, D], f32)
            nc.vector.tensor_scalar_mul(ex2[:], stat_g[:, D:256], 1.0 / N_NODES)
            msq = cp.tile([1, D], f32)
            nc.vector.tensor_tensor(msq[:], mean[:], mean[:], op=OP.mult)
            var = cp.tile([1, D], f32)
            nc.vector.tensor_tensor(var[:], ex2[:], msq[:], op=OP.subtract)
            vpe = cp.tile([1, D], f32)
            nc.vector.tensor_scalar_add(vpe[:], var[:], BN_EPS)
            lnv = cp.tile([1, D], f32)
            nc.scalar.activation(lnv[:], vpe[:], AF.Ln)
            rstd = cp.tile([1, D], f32)
            nc.scalar.activation(rstd[:], lnv[:], AF.Exp, scale=-0.5)
            scale_r = cp.tile([1, D], f32)
            nc.vector.tensor_tensor(scale_r[:], bng_s[:], rstd[:], op=OP.mult)
            mscl = cp.tile([1, D], f32)
            nc.vector.tensor_tensor(mscl[:], mean[:], scale_r[:], op=OP.mult)
            shift_r = cp.tile([1, D], f32)
            nc.vector.tensor_tensor(shift_r[:], bnb_s[:], mscl[:], op=OP.subtract)

            sc_t = cp.tile([P, D], f32)
            nc.gpsimd.partition_broadcast(sc_t[:], scale_r[:])
            sh_t = cp.tile([P, D], f32)
            nc.gpsimd.partition_broadcast(sh_t[:], shift_r[:])

            # residual h via on-chip transpose of hTs; softplus output
            for q in range((NW + 3) // 4):
                q0 = q * 4
                qn = min(4, NW - q0)
                hq = wp.tile([P, 4 * P], f16, tag="hq")
                nc.sync.dma_start(hq[:, :qn * P], hTs[:, q0 * P:(q0 + qn) * P])
                ps_h = ptr.tile([P, 4, D], f16, tag="tr")
                for j in range(qn):
                    nc.tensor.transpose(ps_h[:, j, :], hq[:, j * P:(j + 1) * P], ident[:])
                hres32 = wp.tile([P, 4, D], f32, tag="hres32")
                nc.scalar.copy(hres32[:, :qn, :], ps_h[:, :qn, :])
                t1 = wp.tile([P, 4, D], f32, tag="t1")
                for j in range(qn):
                    nc.vector.tensor_tensor(t1[:, j, :], agg[:, q0 + j, :], sc_t[:], op=OP.mult)
                    nc.vector.tensor_tensor(t1[:, j, :], t1[:, j, :], sh_t[:], op=OP.add)
                nc.vector.tensor_tensor(t1[:, :qn, :], t1[:, :qn, :], hres32[:, :qn, :], op=OP.add)
                t2 = wp.tile([P, 4, D], f32, tag="t2")
                nc.scalar.activation(t2[:, :qn, :], t1[:, :qn, :], AF.Exp)
                t3 = wp.tile([P, 4, D], f32, tag="t3")
                nc.scalar.activation(t3[:, :qn, :], t2[:, :qn, :], AF.Ln, bias=1.0)
                for j in range(qn):
                    nc.sync.dma_start(out_d[(q0 + j) * P:(q0 + j + 1) * P, :], t3[:, j, :])

    nc.compile()
    return nc


_NC_CACHE = None
_NC_LAYOUT = None


def kernel(h, edge_index, edge_feat, gate_w, gate_b, cand_w, cand_b,
           bn_gamma, bn_beta):
    global _NC_CACHE, _NC_LAYOUT
    from concourse.bass_utils import run_bass_kernel_spmd

    h = np.asarray(h, dtype=np.float32)
    ei = np.asarray(edge_index)
    src = ei[0].astype(np.int64)
    dst = ei[1].astype(np.int64)
    ef = np.asarray(edge_feat, dtype=np.float32)
    gw = np.asarray(gate_w, dtype=np.float32)
    gb = np.asarray(gate_b, dtype=np.float32)
    cw = np.asarray(cand_w, dtype=np.float32)
    cb = np.asarray(cand_b, dtype=np.float32)
    gam = np.asarray(bn_gamma, dtype=np.float32).reshape(1, D)
    bet = np.asarray(bn_beta, dtype=np.float32).reshape(1, D)

    wsrc = np.concatenate([-gw[0:D], cw[0:D]], axis=1).astype(np.float16)
    wdst = np.concatenate([-gw[D:2 * D], cw[D:2 * D]], axis=1).astype(np.float16)
    wef_h = np.concatenate([-gw[2 * D:], cw[2 * D:]], axis=1).astype(np.float16)
    bias = np.concatenate([-gb, cb]).astype(np.float32)[None, :].repeat(128, 0)

    h16 = h.astype(np.float16)
    ef_t = ef.T.astype(np.float16)

    LO_T, HI_T, per_core = plan_layout(src, dst)
    layout = (tuple(LO_T), tuple(HI_T))

    in_maps = []
    for k in range(NCORES):
        idx_all_k, eft_k = pack_core(per_core[k], LO_T, HI_T, ef_t)
        h16r = np.zeros((HROWS, D), dtype=np.float16)
        uniq = per_core[k][4]
        h16r[:uniq.shape[0]] = h16[uniq]
        base = k * NB
        hTs16 = np.zeros((D, SROWS), dtype=np.float16)
        hTs16[:, :NB] = h.T[:, base:base + NB].astype(np.float16)
        in_maps.append({
            "h16r": h16r, "hTs": hTs16, "wsrc": wsrc, "wdst": wdst,
            "wef": wef_h, "bias": bias, "eft": eft_k, "idx_all": idx_all_k,
            "bng": gam, "bnb": bet,
        })

    _setup_act_tables()
    if _NC_CACHE is None or _NC_LAYOUT != layout:
        _NC_CACHE = _build_nc(LO_T, HI_T)
        _NC_LAYOUT = layout
    res = run_bass_kernel_spmd(_NC_CACHE, in_maps, core_ids=list(range(NCORES)))
    out = np.concatenate([res.results[k]["out"][:NB] for k in range(NCORES)], axis=0)
    return out.astype(np.float32)


# revision 4
# speedup vs baseline: 1.0139x; 1.0139x over previous
"""CGCNNConv fused kernel v2 — minimal HBM traffic (the device is DMA-BW bound).

Design vs baseline:
- No DRAM tables at all. The only per-edge HBM traffic is ONE gather of the
  raw 256B h[dst] row (f16). dst rows are expanded to the gate/cand
  preactivation on-chip: transpose h_dst tile on PE, matmul with wdst.
- src-side contribution: S_win = h_win @ wsrc + bias computed once per window
  (128 src nodes), expanded per edge with a transposed one-hot matmul.
- Edges sharded by src ownership; per core, dst ids are rank-packed against
  the core's unique-dst table and sorted by (window, rank<25088, rank); the
  lo/hi split keeps gather indices within int16. Pad slots gather row 0 and
  are killed by their -512 one-hot label.
- Scatter-add via one-hot matmul per tile (as baseline), BN stats on-chip,
  1KB AllReduce, residual h recovered from hT by on-chip transpose.
"""

import numpy as np

N_NODES = 50000
N_EDGES = 800000
D = 128
DE = 10
NCORES = 8
NB = N_NODES // NCORES          # 6250 nodes per core
NW = 49                         # windows of 128 src nodes
NLO = 25088                     # lo/hi split of dst index space (196*128)
HROWS = 50176                   # padded h row count (392*128)
SROWS = NW * 128                # 6272 = padded local nodes
BN_EPS = 1e-5


def _wrap_idx(flat16):
    """dma_gather index layout: flat[k] -> partition k%16 (replicated x8), free k//16."""
    n = flat16.shape[0]
    arr = flat16.reshape(n // 16, 16).T          # [16, n/16]
    return np.tile(arr, (8, 1))                  # [128, n/16] int16


def plan_layout(src, dst):
    """Static per-window tile layout shared by all cores.

    Returns (LO_T, HI_T): lists of per-window lo/hi tile counts
    (max over cores, T_w = LO_T+HI_T forced even), plus per-core edge
    membership for packing.
    """
    per_core = []
    lo_need = np.zeros(NW, dtype=np.int64)
    hi_need = np.zeros(NW, dtype=np.int64)
    for k in range(NCORES):
        base = k * NB
        sel = np.where((src >= base) & (src < base + NB))[0]
        s_loc = src[sel] - base
        d = dst[sel]
        # dense per-core packing: gather by rank of dst among this core's
        # unique dsts -> ascending, gap-free gather addresses (row-buffer hits)
        uniq, inv = np.unique(d, return_inverse=True)
        assert uniq.shape[0] <= HROWS
        order = np.lexsort((inv, inv >= NLO, s_loc // 128))
        sel = sel[order]
        s_loc = s_loc[order]
        inv = inv[order]
        win = s_loc // 128
        bounds = np.searchsorted(win, np.arange(NW + 1))
        for w in range(NW):
            lo_, hi_ = bounds[w], bounds[w + 1]
            rw = inv[lo_:hi_]
            nlo = int((rw < NLO).sum())
            nhi = int(rw.shape[0] - nlo)
            lo_need[w] = max(lo_need[w], (nlo + 127) // 128)
            hi_need[w] = max(hi_need[w], (nhi + 127) // 128)
        per_core.append((sel, s_loc, inv, bounds, uniq))
    LO_T = lo_need.copy()
    HI_T = hi_need.copy()
    for w in range(NW):
        if (LO_T[w] + HI_T[w]) % 2 == 1:
            HI_T[w] += 1
    return [int(x) for x in LO_T], [int(x) for x in HI_T], per_core


def pack_core(core_data, LO_T, HI_T, ef_t):
    """Build one core's idx_all [128, CTOT] i32 and eft [11, 128*TT] f16."""
    sel, s_loc, inv, bounds, uniq = core_data
    TT = sum(LO_T) + sum(HI_T)
    eft = np.zeros((11, 128 * TT), dtype=np.float16)
    cols = []
    tile_off = 0
    for w in range(NW):
        lo_, hi_ = bounds[w], bounds[w + 1]
        dw = inv[lo_:hi_]
        sw = s_loc[lo_:hi_] - 128 * w
        ew = sel[lo_:hi_]
        is_lo = dw < NLO
        nlo = int(is_lo.sum())
        nhi = int(dw.shape[0] - nlo)
        LO_CAP = LO_T[w] * 128
        HI_CAP = HI_T[w] * 128
        T_w = LO_T[w] + HI_T[w]
        assert nlo <= LO_CAP and nhi <= HI_CAP

        lo_idx = np.zeros(LO_CAP, dtype=np.int16)
        lo_idx[:nlo] = dw[is_lo].astype(np.int16)
        hi_idx = np.zeros(HI_CAP, dtype=np.int16)
        hi_idx[:nhi] = (dw[~is_lo] - NLO).astype(np.int16)

        wl = np.full(T_w * 128, -512.0, dtype=np.float16)
        wl[:nlo] = sw[is_lo].astype(np.float16)
        wl[LO_CAP:LO_CAP + nhi] = sw[~is_lo].astype(np.float16)

        base_col = 128 * tile_off
        eft[0:DE, base_col:base_col + 128 * T_w][:, :nlo] = ef_t[:, ew[is_lo]]
        eft[0:DE, base_col + LO_CAP:base_col + LO_CAP + nhi] = ef_t[:, ew[~is_lo]]
        eft[DE, base_col:base_col + 128 * T_w] = wl

        wl_p = wl.reshape(T_w, 128).T.astype(np.float16)   # [128, T_w]
        if T_w % 2 == 1:
            wl_p = np.concatenate([wl_p, np.full((128, 1), -512.0, np.float16)], axis=1)
        blk = np.concatenate([
            _wrap_idx(lo_idx).view(np.int32),
            _wrap_idx(hi_idx).view(np.int32),
            np.ascontiguousarray(wl_p).view(np.int32),
        ], axis=1)
        cols.append(blk)
        tile_off += T_w
    idx_all = np.concatenate(cols, axis=1)
    return np.ascontiguousarray(idx_all), eft


def _setup_act_tables():
    """Single combined exp+ln act table set (avoids per-transition reloads)."""
    import os, json, glob, shutil, tempfile
    if os.environ.get("BASS_ACT_ROOT_JSON_PATH"):
        return
    import neuronxcc
    cand = glob.glob(os.path.join(os.path.dirname(neuronxcc.__file__),
                                  "pwp", "pwp_bin_*", "act_info.json"))
    srcj = None
    for c in cand:
        dd = json.load(open(c))
        names = [s.get("name") for s in dd.get("act_func_sets", [])]
        if "natural_log_exp_and_others" in names:
            srcj = c
            break
    if srcj is None:
        return
    dstdir = os.path.join(tempfile.gettempdir(), "act_nlexp_only")
    os.makedirs(dstdir, exist_ok=True)
    dd = json.load(open(srcj))
    keep = [s for s in dd["act_func_sets"] if s["name"] == "natural_log_exp_and_others"]
    dd["act_func_sets"] = keep
    srcdir = os.path.dirname(srcj)
    for s in keep:
        for key in ("bkt_bin", "ctrl_bin", "profile_json"):
            f = s.get(key)
            if f and not os.path.exists(os.path.join(dstdir, f)):
                shutil.copy(os.path.join(srcdir, f), os.path.join(dstdir, f))
    for f in glob.glob(os.path.join(srcdir, "*.bin")) + glob.glob(os.path.join(srcdir, "*.json")):
        b = os.path.basename(f)
        if b != "act_info.json" and not os.path.exists(os.path.join(dstdir, b)):
            try:
                os.symlink(f, os.path.join(dstdir, b))
            except OSError:
                pass
    with open(os.path.join(dstdir, "act_info.json"), "w") as fh:
        json.dump(dd, fh)
    os.environ["BASS_ACT_ROOT_JSON_PATH"] = os.path.join(dstdir, "act_info.json")

    import concourse.hw_specs as hw_specs
    import concourse.bacc as bacc_mod
    import concourse.mybir as mybir
    tables = {keep[0]["name"]: {mybir.ActivationFunctionType.from_pwp(v)
                                for v in keep[0]["act"].keys()}}

    def _patched(module_arch):
        return tables
    hw_specs.get_activation_tables = _patched
    bacc_mod.get_activation_tables = _patched


def _build_nc(LO_T, HI_T):
    import concourse.bass as bass
    import concourse.bacc as bacc
    import concourse.mybir as mybir
    import concourse.tile as tile
    from concourse.masks import make_identity

    f16, f32, i32, i16 = (mybir.dt.float16, mybir.dt.float32,
                          mybir.dt.int32, mybir.dt.int16)
    AF = mybir.ActivationFunctionType
    OP = mybir.AluOpType
    P = 128

    T_W = [LO_T[w] + HI_T[w] for w in range(NW)]
    TT = sum(T_W)
    TMAX = max(T_W)
    # idx_all column offsets per window (int32 cols)
    C_W = [LO_T[w] * 4 + HI_T[w] * 4 + (T_W[w] + 1) // 2 for w in range(NW)]
    COFF = np.concatenate([[0], np.cumsum(C_W)]).astype(int)
    TOFF = np.concatenate([[0], np.cumsum(T_W)]).astype(int)
    CMAX = max(C_W)

    nc = bacc.Bacc("TRN2", target_bir_lowering=False, debug=False,
                   num_devices=NCORES, num_swdge_queues=2)

    h16r = nc.dram_tensor("h16r", [HROWS, D], f16, kind="ExternalInput")
    hTs = nc.dram_tensor("hTs", [P, SROWS], f16, kind="ExternalInput")
    wsrc = nc.dram_tensor("wsrc", [P, 256], f16, kind="ExternalInput")
    wdst = nc.dram_tensor("wdst", [P, 256], f16, kind="ExternalInput")
    wef = nc.dram_tensor("wef", [DE, 256], f16, kind="ExternalInput")
    bias = nc.dram_tensor("bias", [P, 256], f32, kind="ExternalInput")
    eft = nc.dram_tensor("eft", [DE + 1, 128 * TT], f16, kind="ExternalInput")
    idx_all = nc.dram_tensor("idx_all", [P, int(COFF[-1])], i32, kind="ExternalInput")
    bng = nc.dram_tensor("bng", [1, D], f32, kind="ExternalInput")
    bnb = nc.dram_tensor("bnb", [1, D], f32, kind="ExternalInput")
    out_d = nc.dram_tensor("out", [SROWS, D], f32, kind="ExternalOutput")

    with tile.TileContext(nc) as tc:
        with (
            tc.tile_pool(name="const", bufs=1) as cp,
            tc.tile_pool(name="win", bufs=2) as wp,       # per-window big tiles
            tc.tile_pool(name="sm", bufs=4) as sp,        # small per-tile tiles
            tc.tile_pool(name="psPP", bufs=2, space="PSUM") as ppp,   # preact groups
            tc.tile_pool(name="psTR", bufs=2, space="PSUM") as ptr,   # transposes
            tc.tile_pool(name="psS", bufs=1, space="PSUM") as psS,
            tc.tile_pool(name="psB", bufs=1, space="PSUM") as psB,
            tc.tile_pool(name="psW", bufs=2, space="PSUM") as psW,
            tc.tile_pool(name="dram", bufs=1, space="DRAM") as dp,
        ):
            # ---------- constants ----------
            ident = cp.tile([P, P], f16)
            make_identity(nc, ident[:])
            iota_i = cp.tile([P, P], i16)
            nc.gpsimd.iota(iota_i[:], pattern=[[1, P]], base=0, channel_multiplier=0)
            iota_f = cp.tile([P, P], f16)
            nc.vector.tensor_copy(iota_f[:], iota_i[:])
            iotap_i = cp.tile([P, 1], i16)
            nc.gpsimd.iota(iotap_i[:], pattern=[[1, 1]], base=0, channel_multiplier=1)
            iotap_f = cp.tile([P, 1], f32)
            nc.vector.tensor_copy(iotap_f[:], iotap_i[:])
            ones1 = cp.tile([1, P], f16)
            nc.vector.memset(ones1[:], 1.0)
            ones_c = cp.tile([P, 1], f32)
            nc.vector.memset(ones_c[:], 1.0)

            wsrc_s = cp.tile([P, 256], f16)
            nc.sync.dma_start(wsrc_s[:], wsrc[:])
            wdst_s = cp.tile([P, 256], f16)
            nc.sync.dma_start(wdst_s[:], wdst[:])
            wef_s = cp.tile([DE, 256], f16)
            nc.sync.dma_start(wef_s[:], wef[:])
            bias_s = cp.tile([P, 256], f32)
            nc.sync.dma_start(bias_s[:], bias[:])

            agg = cp.tile([P, NW, D], f32)
            rstat = cp.tile([P, 256], f32)
            nc.vector.memset(rstat[:], 0.0)

            # pre-clear both rotation buffers of the gather destination
            # (trimmed trailing pad indices leave slots unwritten; initial
            # SBUF garbage could be NaN and would poison 0*NaN in matmuls)
            for _ in range(2):
                hd0 = wp.tile([P, TMAX, D], f16, tag="hdst")
                nc.vector.memset(hd0[:], 0.0)

            # ---------- main edge loop ----------
            for w in range(NW):
                lo_t, hi_t, t_w = LO_T[w], HI_T[w], T_W[w]
                lo_cap, hi_cap = lo_t * 128, hi_t * 128
                ng = t_w // 2

                ia = wp.tile([P, CMAX], i32, tag="ia")
                nc.sync.dma_start(ia[:, :C_W[w]], idx_all[:, int(COFF[w]):int(COFF[w + 1])])
                li = ia[:, 0:lo_t * 4]
                hi_ = ia[:, lo_t * 4:lo_t * 4 + hi_t * 4]
                wl = ia[:, lo_t * 4 + hi_t * 4:C_W[w]].bitcast(f16)

                efts = wp.tile([DE, TMAX * 128], f16, tag="efts")
                nc.sync.dma_start(efts[:, :t_w * 128],
                                  eft[0:DE, int(TOFF[w]) * 128:int(TOFF[w + 1]) * 128])
                wlr = wp.tile([1, TMAX * 128], f16, tag="wlr")
                nc.sync.dma_start(wlr[:, :t_w * 128],
                                  eft[DE:DE + 1, int(TOFF[w]) * 128:int(TOFF[w + 1]) * 128])

                # S_win = h_win @ wsrc + bias  (on-chip, f16)
                hw_t = sp.tile([P, P], f16, tag="hwt")
                nc.sync.dma_start(hw_t[:], hTs[:, w * P:(w + 1) * P])
                ps_s = psS.tile([P, 256], f32, tag="swin")
                nc.tensor.matmul(ps_s[:], lhsT=hw_t[:], rhs=wsrc_s[:], start=True, stop=True)
                s16 = sp.tile([P, 256], f16, tag="s16")
                nc.vector.tensor_tensor(s16[:], ps_s[:], bias_s[:], op=OP.add)

                # gather h[dst] rows (256B each); trailing -1 idx are trimmed
                hdst = wp.tile([P, TMAX, D], f16, tag="hdst")
                nc.gpsimd.dma_gather(hdst[:, 0:lo_t, :], h16r[0:NLO, :],
                                     li.bitcast(i16), lo_cap, lo_cap, D,
                                     single_packet=False, queue_num=0)
                nc.gpsimd.dma_gather(hdst[:, lo_t:t_w, :], h16r[NLO:HROWS, :],
                                     hi_.bitcast(i16), hi_cap, hi_cap, D,
                                     single_packet=False, queue_num=1)

                # transposed one-hot (node -> edge) for the S expansion:
                # bcast wl along partitions via K=1 matmul, compare to iota_p
                ohT = wp.tile([P, TMAX * 128], f16, tag="ohT")
                nchunk = (t_w * 128 + 511) // 512
                for c in range(nchunk):
                    c0 = c * 512
                    c1 = min(t_w * 128, c0 + 512)
                    ps_b = psB.tile([P, 512], f32, tag="bcast")
                    nc.tensor.matmul(ps_b[:, :c1 - c0], lhsT=ones1[:],
                                     rhs=wlr[:, c0:c1], start=True, stop=True)
                    nc.vector.tensor_tensor(ohT[:, c0:c1],
                                            iotap_f[:].to_broadcast([P, c1 - c0]),
                                            ps_b[:, :c1 - c0], op=OP.is_equal)

                # transpose gathered h rows: [edge, feat] -> [feat, edge]
                hdT = wp.tile([P, TMAX, D], f16, tag="hdT")
                for q in range((t_w + 3) // 4):
                    q0 = q * 4
                    qn = min(4, t_w - q0)
                    ps_t = ptr.tile([P, 4, D], f16, tag="tr")
                    for j in range(qn):
                        nc.tensor.transpose(ps_t[:, j, :], hdst[:, q0 + j, :], ident[:])
                    nc.scalar.copy(hdT[:, q0:q0 + qn, :], ps_t[:, :qn, :])

                # preact per 2-tile group, act path
                e16 = wp.tile([P, TMAX * 256], f16, tag="e16")
                for g in range(ng):
                    t0 = 2 * g
                    pp = ppp.tile([P, 2, 256], f32, tag="pp")
                    for j in range(2):
                        t = t0 + j
                        nc.tensor.matmul(pp[:, j, :], lhsT=efts[:, t * 128:(t + 1) * 128],
                                         rhs=wef_s[:], start=True, stop=False)
                        nc.tensor.matmul(pp[:, j, :], lhsT=hdT[:, t, :],
                                         rhs=wdst_s[:], start=False, stop=False)
                        nc.tensor.matmul(pp[:, j, :], lhsT=ohT[:, t * 128:(t + 1) * 128],
                                         rhs=s16[:], start=False, stop=True)
                    nc.scalar.activation(e16[:, g * 512:(g + 1) * 512], pp[:], AF.Exp)

                u16 = wp.tile([P, TMAX * 256], f16, tag="u16")
                nc.scalar.activation(u16[:, :ng * 512], e16[:, :ng * 512], AF.Ln, bias=1.0)

                m16 = wp.tile([P, TMAX, D], f16, tag="m16")
                for g in range(ng):
                    g16 = sp.tile([P, 2, D], f16, tag="g16")
                    nc.scalar.activation(
                        g16[:], u16[:, g * 512:g * 512 + 512].rearrange("a (b c) -> a b c", b=2)[:, :, 0:D],
                        AF.Exp, scale=-1.0)
                    nc.vector.tensor_tensor(
                        m16[:, 2 * g:2 * g + 2, :], g16[:],
                        u16[:, g * 512:g * 512 + 512].rearrange("a (b c) -> a b c", b=2)[:, :, D:256],
                        op=OP.mult)

                # scatter-add via one-hot matmuls
                pw = psW.tile([P, D], f32, tag="winps")
                for t in range(t_w):
                    oh = sp.tile([P, P], f16, tag="oh")
                    nc.vector.tensor_tensor(oh[:], iota_f[:],
                                            wl[:, t:t + 1].to_broadcast([P, P]),
                                            op=OP.is_equal)
                    nc.tensor.matmul(pw[:], lhsT=oh[:], rhs=m16[:, t, :],
                                     start=(t == 0), stop=(t == t_w - 1))

                nc.vector.tensor_copy(agg[:, w, :], pw[:])
                sq = sp.tile([P, D], f32, tag="sq")
                nc.vector.tensor_tensor(sq[:], agg[:, w, :], agg[:, w, :], op=OP.mult)
                nc.vector.tensor_tensor(rstat[:, 0:D], rstat[:, 0:D], agg[:, w, :], op=OP.add)
                nc.vector.tensor_tensor(rstat[:, D:256], rstat[:, D:256], sq[:], op=OP.add)

            # ---------- BN stats + output ----------
            pstat = psS.tile([1, 256], f32, tag="swin")
            nc.tensor.matmul(pstat[:], lhsT=ones_c[:], rhs=rstat[:], start=True, stop=True)
            stat_l = cp.tile([1, 256], f32)
            nc.vector.tensor_copy(stat_l[:], pstat[:])

            cc_in = dp.tile([1, 256], f32)
            cc_out = dp.tile([1, 256], f32)
            nc.gpsimd.dma_start(cc_in[:], stat_l[:])
            nc.gpsimd.collective_compute(
                "AllReduce", mybir.AluOpType.add,
                replica_groups=[list(range(NCORES))],
                ins=[cc_in.opt()], outs=[cc_out.opt()])
            stat_g = cp.tile([1, 256], f32)
            nc.sync.dma_start(stat_g[:], cc_out[:])

            bng_s = cp.tile([1, D], f32)
            nc.sync.dma_start(bng_s[:], bng[:])
            bnb_s = cp.tile([1, D], f32)
            nc.sync.dma_start(bnb_s[:], bnb[:])

            mean = cp.tile([1, D], f32)
            nc.vector.tensor_scalar_mul(mean[:], stat_g[:, 0:D], 1.0 / N_NODES)
            ex2 = cp.tile([1, D], f32)
            nc.vector.tensor_scalar_mul(ex2[:], stat_g[:, D:256], 1.0 / N_NODES)
            msq = cp.tile([1, D], f32)
            nc.vector.tensor_tensor(msq[:], mean[:], mean[:], op=OP.mult)
            var = cp.tile([1, D], f32)
            nc.vector.tensor_tensor(var[:], ex2[:], msq[:], op=OP.subtract)
            vpe = cp.tile([1, D], f32)
            nc.vector.tensor_scalar_add(vpe[:], var[:], BN_EPS)
            lnv = cp.tile([1, D], f32)
            nc.scalar.activation(lnv[:], vpe[:], AF.Ln)
            rstd = cp.tile([1, D], f32)
            nc.scalar.activation(rstd[:], lnv[:], AF.Exp, scale=-0.5)
            scale_r = cp.tile([1, D], f32)
            nc.vector.tensor_tensor(scale_r[:], bng_s[:], rstd[:], op=OP.mult)
            mscl = cp.tile([1, D], f32)
            nc.vector.tensor_tensor(mscl[:], mean[:], scale_r[:], op=OP.mult)
            shift_r = cp.tile([1, D], f32)
            nc.vector.tensor_tensor(shift_r[:], bnb_s[:], mscl[:], op=OP.subtract)

            sc_t = cp.tile([P, D], f32)
            nc.gpsimd.partition_broadcast(sc_t[:], scale_r[:])
            sh_t = cp.tile([P, D], f32)
            nc.gpsimd.partition_broadcast(sh_t[:], shift_r[:])

            # residual h via on-chip transpose of hTs; softplus output
            for q in range((NW + 3) // 4):
                q0 = q * 4
                qn = min(4, NW - q0)
                hq = wp.tile([P, 4 * P], f16, tag="hq")
                nc.sync.dma_start(hq[:, :qn * P], hTs[:, q0 * P:(q0 + qn) * P])
                ps_h = ptr.tile([P, 4, D], f16, tag="tr")
                for j in range(qn):
                    nc.tensor.transpose(ps_h[:, j, :], hq[:, j * P:(j + 1) * P], ident[:])
                hres32 = wp.tile([P, 4, D], f32, tag="hres32")
                nc.scalar.copy(hres32[:, :qn, :], ps_h[:, :qn, :])
                t1 = wp.tile([P, 4, D], f32, tag="t1")
                for j in range(qn):
                    nc.vector.tensor_tensor(t1[:, j, :], agg[:, q0 + j, :], sc_t[:], op=OP.mult)
                    nc.vector.tensor_tensor(t1[:, j, :], t1[:, j, :], sh_t[:], op=OP.add)
                nc.vector.tensor_tensor(t1[:, :qn, :], t1[:, :qn, :], hres32[:, :qn, :], op=OP.add)
                t2 = wp.tile([P, 4, D], f32, tag="t2")
                nc.scalar.activation(t2[:, :qn, :], t1[:, :qn, :], AF.Exp)
                t3 = wp.tile([P, 4, D], f32, tag="t3")
                nc.scalar.activation(t3[:, :qn, :], t2[:, :qn, :], AF.Ln, bias=1.0)
                for j in range(qn):
                    nc.sync.dma_start(out_d[(q0 + j) * P:(q0 + j + 1) * P, :], t3[:, j, :])

    nc.compile()
    return nc


_NC_CACHE = None
_NC_LAYOUT = None


def kernel(h, edge_index, edge_feat, gate_w, gate_b, cand_w, cand_b,
           bn_gamma, bn_beta):
    global _NC_CACHE, _NC_LAYOUT
    from concourse.bass_utils import run_bass_kernel_spmd

    h = np.asarray(h, dtype=np.float32)
    ei = np.asarray(edge_index)
    src = ei[0].astype(np.int64)
    dst = ei[1].astype(np.int64)
    ef = np.asarray(edge_feat, dtype=np.float32)
    gw = np.asarray(gate_w, dtype=np.float32)
    gb = np.asarray(gate_b, dtype=np.float32)
    cw = np.asarray(cand_w, dtype=np.float32)
    cb = np.asarray(cand_b, dtype=np.float32)
    gam = np.asarray(bn_gamma, dtype=np.float32).reshape(1, D)
    bet = np.asarray(bn_beta, dtype=np.float32).reshape(1, D)

    wsrc = np.concatenate([-gw[0:D], cw[0:D]], axis=1).astype(np.float16)
    wdst = np.concatenate([-gw[D:2 * D], cw[D:2 * D]], axis=1).astype(np.float16)
    wef_h = np.concatenate([-gw[2 * D:], cw[2 * D:]], axis=1).astype(np.float16)
    bias = np.concatenate([-gb, cb]).astype(np.float32)[None, :].repeat(128, 0)

    h16 = h.astype(np.float16)
    ef_t = ef.T.astype(np.float16)

    LO_T, HI_T, per_core = plan_layout(src, dst)
    layout = (tuple(LO_T), tuple(HI_T))

    in_maps = []
    for k in range(NCORES):
        idx_all_k, eft_k = pack_core(per_core[k], LO_T, HI_T, ef_t)
        h16r = np.zeros((HROWS, D), dtype=np.float16)
        uniq = per_core[k][4]
        h16r[:uniq.shape[0]] = h16[uniq]
        base = k * NB
        hTs16 = np.zeros((D, SROWS), dtype=np.float16)
        hTs16[:, :NB] = h.T[:, base:base + NB].astype(np.float16)
        in_maps.append({
            "h16r": h16r, "hTs": hTs16, "wsrc": wsrc, "wdst": wdst,
            "wef": wef_h, "bias": bias, "eft": eft_k, "idx_all": idx_all_k,
            "bng": gam, "bnb": bet,
        })

    _setup_act_tables()
    if _NC_CACHE is None or _NC_LAYOUT != layout:
        _NC_CACHE = _build_nc(LO_T, HI_T)
        _NC_LAYOUT = layout
    res = run_bass_kernel_spmd(_NC_CACHE, in_maps, core_ids=list(range(NCORES)))
    out = np.concatenate([res.results[k]["out"][:NB] for k in range(NCORES)], axis=0)
    return out.astype(np.float32)


# revision 5
# speedup vs baseline: 1.0383x; 1.0241x over previous
"""CGCNNConv fused kernel v2 — minimal HBM traffic (the device is DMA-BW bound).

Design vs baseline:
- No DRAM tables at all. The only per-edge HBM traffic is ONE gather of the
  raw 256B h[dst] row (f16). dst rows are expanded to the gate/cand
  preactivation on-chip: transpose h_dst tile on PE, matmul with wdst.
- src-side contribution: S_win = h_win @ wsrc + bias computed once per window
  (128 src nodes), expanded per edge with a transposed one-hot matmul.
- Edges sharded by src ownership; per core, dst ids are rank-packed against
  the core's unique-dst table and sorted by (window, rank<25088, rank); the
  lo/hi split keeps gather indices within int16. Pad slots gather row 0 and
  are killed by their -512 one-hot label.
- Scatter-add via one-hot matmul per tile (as baseline), BN stats on-chip,
  1KB AllReduce, residual h recovered from hT by on-chip transpose.

Measured cost model (slope-timed: chained executes, marginal ns/iter; the
single-shot wall clock carries ~70 ms of axon-tunnel overhead):
- this kernel: 16.4-16.7 ms/iter (baseline table kernel: 22.3 ms/iter)
- sequential DMA: ~10 GB/s effective per (virtualized) core
- dma_gather: bound by per-descriptor latency (~110 ns/desc across engines),
  INDEPENDENT of descriptor size (256B vs 512B pair-rows identical) and of
  address locality (dense rank-packed vs raw node ids identical)
- instructions are nearly free (2000 matmul/DVE/ACT ops ~ 0.2 ms)
Known device-wedging traps (hang at result fetch, needs minutes to recover):
trailing -1 gather indices (decode reserves ring space pre-trim, Q7 writes
post-trim) and single_packet=True.
Next win (unimplemented): quad-row dedup -- gather unique 1KB quads of the
rank-packed table (~34k descs/core vs ~112k), expand quads->edges with a
one-hot matmul; requires repacking so each 128-edge tile references one
128-quad tile. Projected ~8 ms.
"""

import numpy as np

N_NODES = 50000
N_EDGES = 800000
D = 128
DE = 10
NCORES = 8
NB = N_NODES // NCORES          # 6250 nodes per core
NW = 49                         # windows of 128 src nodes
NLO = 25088                     # lo/hi split of dst index space (196*128)
HROWS = 50176                   # padded h row count (392*128)
SROWS = NW * 128                # 6272 = padded local nodes
BN_EPS = 1e-5


def _wrap_idx(flat16):
    """dma_gather index layout: flat[k] -> partition k%16 (replicated x8), free k//16."""
    n = flat16.shape[0]
    arr = flat16.reshape(n // 16, 16).T          # [16, n/16]
    return np.tile(arr, (8, 1))                  # [128, n/16] int16


def plan_layout(src, dst):
    """Static per-window tile layout shared by all cores.

    Returns (LO_T, HI_T): lists of per-window lo/hi tile counts
    (max over cores, T_w = LO_T+HI_T forced even), plus per-core edge
    membership for packing.
    """
    per_core = []
    lo_need = np.zeros(NW, dtype=np.int64)
    hi_need = np.zeros(NW, dtype=np.int64)
    for k in range(NCORES):
        base = k * NB
        sel = np.where((src >= base) & (src < base + NB))[0]
        s_loc = src[sel] - base
        d = dst[sel]
        # dense per-core packing: gather by rank of dst among this core's
        # unique dsts -> ascending, gap-free gather addresses (row-buffer hits)
        uniq, inv = np.unique(d, return_inverse=True)
        assert uniq.shape[0] <= HROWS
        order = np.lexsort((inv, inv >= NLO, s_loc // 128))
        sel = sel[order]
        s_loc = s_loc[order]
        inv = inv[order]
        win = s_loc // 128
        bounds = np.searchsorted(win, np.arange(NW + 1))
        for w in range(NW):
            lo_, hi_ = bounds[w], bounds[w + 1]
            rw = inv[lo_:hi_]
            nlo = int((rw < NLO).sum())
            nhi = int(rw.shape[0] - nlo)
            lo_need[w] = max(lo_need[w], (nlo + 127) // 128)
            hi_need[w] = max(hi_need[w], (nhi + 127) // 128)
        per_core.append((sel, s_loc, inv, bounds, uniq))
    LO_T = lo_need.copy()
    HI_T = hi_need.copy()
    for w in range(NW):
        if (LO_T[w] + HI_T[w]) % 2 == 1:
            HI_T[w] += 1
    return [int(x) for x in LO_T], [int(x) for x in HI_T], per_core


def pack_core(core_data, LO_T, HI_T, ef_t):
    """Build one core's idx_all [128, CTOT] i32 and eft [11, 128*TT] f16."""
    sel, s_loc, inv, bounds, uniq = core_data
    TT = sum(LO_T) + sum(HI_T)
    eft = np.zeros((11, 128 * TT), dtype=np.float16)
    cols = []
    tile_off = 0
    for w in range(NW):
        lo_, hi_ = bounds[w], bounds[w + 1]
        dw = inv[lo_:hi_]
        sw = s_loc[lo_:hi_] - 128 * w
        ew = sel[lo_:hi_]
        is_lo = dw < NLO
        nlo = int(is_lo.sum())
        nhi = int(dw.shape[0] - nlo)
        LO_CAP = LO_T[w] * 128
        HI_CAP = HI_T[w] * 128
        T_w = LO_T[w] + HI_T[w]
        assert nlo <= LO_CAP and nhi <= HI_CAP

        lo_idx = np.zeros(LO_CAP, dtype=np.int16)
        lo_idx[:nlo] = dw[is_lo].astype(np.int16)
        hi_idx = np.zeros(HI_CAP, dtype=np.int16)
        hi_idx[:nhi] = (dw[~is_lo] - NLO).astype(np.int16)

        wl = np.full(T_w * 128, -512.0, dtype=np.float16)
        wl[:nlo] = sw[is_lo].astype(np.float16)
        wl[LO_CAP:LO_CAP + nhi] = sw[~is_lo].astype(np.float16)

        base_col = 128 * tile_off
        eft[0:DE, base_col:base_col + 128 * T_w][:, :nlo] = ef_t[:, ew[is_lo]]
        eft[0:DE, base_col + LO_CAP:base_col + LO_CAP + nhi] = ef_t[:, ew[~is_lo]]
        eft[DE, base_col:base_col + 128 * T_w] = wl

        wl_p = wl.reshape(T_w, 128).T.astype(np.float16)   # [128, T_w]
        if T_w % 2 == 1:
            wl_p = np.concatenate([wl_p, np.full((128, 1), -512.0, np.float16)], axis=1)
        blk = np.concatenate([
            _wrap_idx(lo_idx).view(np.int32),
            _wrap_idx(hi_idx).view(np.int32),
            np.ascontiguousarray(wl_p).view(np.int32),
        ], axis=1)
        cols.append(blk)
        tile_off += T_w
    idx_all = np.concatenate(cols, axis=1)
    return np.ascontiguousarray(idx_all), eft


def _setup_act_tables():
    """Single combined exp+ln act table set (avoids per-transition reloads)."""
    import os, json, glob, shutil, tempfile
    if os.environ.get("BASS_ACT_ROOT_JSON_PATH"):
        return
    import neuronxcc
    cand = glob.glob(os.path.join(os.path.dirname(neuronxcc.__file__),
                                  "pwp", "pwp_bin_*", "act_info.json"))
    srcj = None
    for c in cand:
        dd = json.load(open(c))
        names = [s.get("name") for s in dd.get("act_func_sets", [])]
        if "natural_log_exp_and_others" in names:
            srcj = c
            break
    if srcj is None:
        return
    dstdir = os.path.join(tempfile.gettempdir(), "act_nlexp_only")
    os.makedirs(dstdir, exist_ok=True)
    dd = json.load(open(srcj))
    keep = [s for s in dd["act_func_sets"] if s["name"] == "natural_log_exp_and_others"]
    dd["act_func_sets"] = keep
    srcdir = os.path.dirname(srcj)
    for s in keep:
        for key in ("bkt_bin", "ctrl_bin", "profile_json"):
            f = s.get(key)
            if f and not os.path.exists(os.path.join(dstdir, f)):
                shutil.copy(os.path.join(srcdir, f), os.path.join(dstdir, f))
    for f in glob.glob(os.path.join(srcdir, "*.bin")) + glob.glob(os.path.join(srcdir, "*.json")):
        b = os.path.basename(f)
        if b != "act_info.json" and not os.path.exists(os.path.join(dstdir, b)):
            try:
                os.symlink(f, os.path.join(dstdir, b))
            except OSError:
                pass
    with open(os.path.join(dstdir, "act_info.json"), "w") as fh:
        json.dump(dd, fh)
    os.environ["BASS_ACT_ROOT_JSON_PATH"] = os.path.join(dstdir, "act_info.json")

    import concourse.hw_specs as hw_specs
    import concourse.bacc as bacc_mod
    import concourse.mybir as mybir
    tables = {keep[0]["name"]: {mybir.ActivationFunctionType.from_pwp(v)
                                for v in keep[0]["act"].keys()}}

    def _patched(module_arch):
        return tables
    hw_specs.get_activation_tables = _patched
    bacc_mod.get_activation_tables = _patched


def _build_nc(LO_T, HI_T):
    import concourse.bass as bass
    import concourse.bacc as bacc
    import concourse.mybir as mybir
    import concourse.tile as tile
    from concourse.masks import make_identity

    f16, f32, i32, i16 = (mybir.dt.float16, mybir.dt.float32,
                          mybir.dt.int32, mybir.dt.int16)
    AF = mybir.ActivationFunctionType
    OP = mybir.AluOpType
    P = 128

    T_W = [LO_T[w] + HI_T[w] for w in range(NW)]
    TT = sum(T_W)
    TMAX = max(T_W)
    # idx_all column offsets per window (int32 cols)
    C_W = [LO_T[w] * 4 + HI_T[w] * 4 + (T_W[w] + 1) // 2 for w in range(NW)]
    COFF = np.concatenate([[0], np.cumsum(C_W)]).astype(int)
    TOFF = np.concatenate([[0], np.cumsum(T_W)]).astype(int)
    CMAX = max(C_W)

    nc = bacc.Bacc("TRN2", target_bir_lowering=False, debug=False,
                   num_devices=NCORES, num_swdge_queues=2)

    h16r = nc.dram_tensor("h16r", [HROWS, D], f16, kind="ExternalInput")
    hTs = nc.dram_tensor("hTs", [P, SROWS], f16, kind="ExternalInput")
    wsrc = nc.dram_tensor("wsrc", [P, 256], f16, kind="ExternalInput")
    wdst = nc.dram_tensor("wdst", [P, 256], f16, kind="ExternalInput")
    wef = nc.dram_tensor("wef", [DE, 256], f16, kind="ExternalInput")
    bias = nc.dram_tensor("bias", [P, 256], f32, kind="ExternalInput")
    eft = nc.dram_tensor("eft", [DE + 1, 128 * TT], f16, kind="ExternalInput")
    idx_all = nc.dram_tensor("idx_all", [P, int(COFF[-1])], i32, kind="ExternalInput")
    bng = nc.dram_tensor("bng", [1, D], f32, kind="ExternalInput")
    bnb = nc.dram_tensor("bnb", [1, D], f32, kind="ExternalInput")
    out_d = nc.dram_tensor("out", [SROWS, D], f32, kind="ExternalOutput")

    with tile.TileContext(nc) as tc:
        with (
            tc.tile_pool(name="const", bufs=1) as cp,
            tc.tile_pool(name="win", bufs=2) as wp,       # per-window big tiles
            tc.tile_pool(name="sm", bufs=4) as sp,        # small per-tile tiles
            tc.tile_pool(name="psPP", bufs=2, space="PSUM") as ppp,   # preact groups
            tc.tile_pool(name="psTR", bufs=2, space="PSUM") as ptr,   # transposes
            tc.tile_pool(name="psS", bufs=1, space="PSUM") as psS,
            tc.tile_pool(name="psB", bufs=1, space="PSUM") as psB,
            tc.tile_pool(name="psW", bufs=2, space="PSUM") as psW,
            tc.tile_pool(name="dram", bufs=1, space="DRAM") as dp,
        ):
            # ---------- constants ----------
            ident = cp.tile([P, P], f16)
            make_identity(nc, ident[:])
            iota_i = cp.tile([P, P], i16)
            nc.gpsimd.iota(iota_i[:], pattern=[[1, P]], base=0, channel_multiplier=0)
            iota_f = cp.tile([P, P], f16)
            nc.vector.tensor_copy(iota_f[:], iota_i[:])
            iotap_i = cp.tile([P, 1], i16)
            nc.gpsimd.iota(iotap_i[:], pattern=[[1, 1]], base=0, channel_multiplier=1)
            iotap_f = cp.tile([P, 1], f32)
            nc.vector.tensor_copy(iotap_f[:], iotap_i[:])
            ones1 = cp.tile([1, P], f16)
            nc.vector.memset(ones1[:], 1.0)
            ones_c = cp.tile([P, 1], f32)
            nc.vector.memset(ones_c[:], 1.0)

            wsrc_s = cp.tile([P, 256], f16)
            nc.sync.dma_start(wsrc_s[:], wsrc[:])
            wdst_s = cp.tile([P, 256], f16)
            nc.sync.dma_start(wdst_s[:], wdst[:])
            wef_s = cp.tile([DE, 256], f16)
            nc.sync.dma_start(wef_s[:], wef[:])
            bias_s = cp.tile([P, 256], f32)
            nc.sync.dma_start(bias_s[:], bias[:])

            agg = cp.tile([P, NW, D], f32)
            rstat = cp.tile([P, 256], f32)
            nc.vector.memset(rstat[:], 0.0)

            # pre-clear both rotation buffers of the gather destination
            # (trimmed trailing pad indices leave slots unwritten; initial
            # SBUF garbage could be NaN and would poison 0*NaN in matmuls)
            for _ in range(2):
                hd0 = wp.tile([P, TMAX, D], f16, tag="hdst")
                nc.vector.memset(hd0[:], 0.0)

            # ---------- main edge loop ----------
            for w in range(NW):
                lo_t, hi_t, t_w = LO_T[w], HI_T[w], T_W[w]
                lo_cap, hi_cap = lo_t * 128, hi_t * 128
                ng = t_w // 2

                ia = wp.tile([P, CMAX], i32, tag="ia")
                nc.sync.dma_start(ia[:, :C_W[w]], idx_all[:, int(COFF[w]):int(COFF[w + 1])])
                li = ia[:, 0:lo_t * 4]
                hi_ = ia[:, lo_t * 4:lo_t * 4 + hi_t * 4]
                wl = ia[:, lo_t * 4 + hi_t * 4:C_W[w]].bitcast(f16)

                efts = wp.tile([DE, TMAX * 128], f16, tag="efts")
                nc.sync.dma_start(efts[:, :t_w * 128],
                                  eft[0:DE, int(TOFF[w]) * 128:int(TOFF[w + 1]) * 128])
                wlr = wp.tile([1, TMAX * 128], f16, tag="wlr")
                nc.sync.dma_start(wlr[:, :t_w * 128],
                                  eft[DE:DE + 1, int(TOFF[w]) * 128:int(TOFF[w + 1]) * 128])

                # S_win = h_win @ wsrc + bias  (on-chip, f16)
                hw_t = sp.tile([P, P], f16, tag="hwt")
                nc.sync.dma_start(hw_t[:], hTs[:, w * P:(w + 1) * P])
                ps_s = psS.tile([P, 256], f32, tag="swin")
                nc.tensor.matmul(ps_s[:], lhsT=hw_t[:], rhs=wsrc_s[:], start=True, stop=True)
                s16 = sp.tile([P, 256], f16, tag="s16")
                nc.vector.tensor_tensor(s16[:], ps_s[:], bias_s[:], op=OP.add)

                # gather h[dst] rows (256B each); trailing -1 idx are trimmed
                hdst = wp.tile([P, TMAX, D], f16, tag="hdst")
                nc.gpsimd.dma_gather(hdst[:, 0:lo_t, :], h16r[0:NLO, :],
                                     li.bitcast(i16), lo_cap, lo_cap, D,
                                     single_packet=False, queue_num=0)
                nc.gpsimd.dma_gather(hdst[:, lo_t:t_w, :], h16r[NLO:HROWS, :],
                                     hi_.bitcast(i16), hi_cap, hi_cap, D,
                                     single_packet=False, queue_num=1)

                # transposed one-hot (node -> edge) for the S expansion:
                # bcast wl along partitions via K=1 matmul, compare to iota_p
                ohT = wp.tile([P, TMAX * 128], f16, tag="ohT")
                nchunk = (t_w * 128 + 511) // 512
                for c in range(nchunk):
                    c0 = c * 512
                    c1 = min(t_w * 128, c0 + 512)
                    ps_b = psB.tile([P, 512], f32, tag="bcast")
                    nc.tensor.matmul(ps_b[:, :c1 - c0], lhsT=ones1[:],
                                     rhs=wlr[:, c0:c1], start=True, stop=True)
                    nc.vector.tensor_tensor(ohT[:, c0:c1],
                                            iotap_f[:].to_broadcast([P, c1 - c0]),
                                            ps_b[:, :c1 - c0], op=OP.is_equal)

                # transpose gathered h rows: [edge, feat] -> [feat, edge]
                hdT = wp.tile([P, TMAX, D], f16, tag="hdT")
                for q in range((t_w + 3) // 4):
                    q0 = q * 4
                    qn = min(4, t_w - q0)
                    ps_t = ptr.tile([P, 4, D], f16, tag="tr")
                    for j in range(qn):
                        nc.tensor.transpose(ps_t[:, j, :], hdst[:, q0 + j, :], ident[:])
                    nc.scalar.copy(hdT[:, q0:q0 + qn, :], ps_t[:, :qn, :])

                # preact per 2-tile group, act path
                e16 = wp.tile([P, TMAX * 256], f16, tag="e16")
                for g in range(ng):
                    t0 = 2 * g
                    pp = ppp.tile([P, 2, 256], f32, tag="pp")
                    for j in range(2):
                        t = t0 + j
                        nc.tensor.matmul(pp[:, j, :], lhsT=efts[:, t * 128:(t + 1) * 128],
                                         rhs=wef_s[:], start=True, stop=False)
                        nc.tensor.matmul(pp[:, j, :], lhsT=hdT[:, t, :],
                                         rhs=wdst_s[:], start=False, stop=False)
                        nc.tensor.matmul(pp[:, j, :], lhsT=ohT[:, t * 128:(t + 1) * 128],
                                         rhs=s16[:], start=False, stop=True)
                    nc.scalar.activation(e16[:, g * 512:(g + 1) * 512], pp[:], AF.Exp)

                u16 = wp.tile([P, TMAX * 256], f16, tag="u16")
                nc.scalar.activation(u16[:, :ng * 512], e16[:, :ng * 512], AF.Ln, bias=1.0)

                m16 = wp.tile([P, TMAX, D], f16, tag="m16")
                for g in range(ng):
                    g16 = sp.tile([P, 2, D], f16, tag="g16")
                    nc.scalar.activation(
                        g16[:], u16[:, g * 512:g * 512 + 512].rearrange("a (b c) -> a b c", b=2)[:, :, 0:D],
                        AF.Exp, scale=-1.0)
                    nc.vector.tensor_tensor(
                        m16[:, 2 * g:2 * g + 2, :], g16[:],
                        u16[:, g * 512:g * 512 + 512].rearrange("a (b c) -> a b c", b=2)[:, :, D:256],
                        op=OP.mult)

                # scatter-add via one-hot matmuls
                pw = psW.tile([P, D], f32, tag="winps")
                for t in range(t_w):
                    oh = sp.tile([P, P], f16, tag="oh")
                    nc.vector.tensor_tensor(oh[:], iota_f[:],
                                            wl[:, t:t + 1].to_broadcast([P, P]),
                                            op=OP.is_equal)
                    nc.tensor.matmul(pw[:], lhsT=oh[:], rhs=m16[:, t, :],
                                     start=(t == 0), stop=(t == t_w - 1))

                nc.vector.tensor_copy(agg[:, w, :], pw[:])
                sq = sp.tile([P, D], f32, tag="sq")
                nc.vector.tensor_tensor(sq[:], agg[:, w, :], agg[:, w, :], op=OP.mult)
                nc.vector.tensor_tensor(rstat[:, 0:D], rstat[:, 0:D], agg[:, w, :], op=OP.add)
                nc.vector.tensor_tensor(rstat[:, D:256], rstat[:, D:256], sq[:], op=OP.add)

            # ---------- BN stats + output ----------
            pstat = psS.tile([1, 256], f32, tag="swin")
            nc.tensor.matmul(pstat[:], lhsT=ones_c[:], rhs=rstat[:], start=True, stop=True)
            stat_l = cp.tile([1, 256], f32)
            nc.vector.tensor_copy(stat_l[:], pstat[:])

            cc_in = dp.tile([1, 256], f32)
            cc_out = dp.tile([1, 256], f32)
            nc.gpsimd.dma_start(cc_in[:], stat_l[:])
            nc.gpsimd.collective_compute(
                "AllReduce", mybir.AluOpType.add,
                replica_groups=[list(range(NCORES))],
                ins=[cc_in.opt()], outs=[cc_out.opt()])
            stat_g = cp.tile([1, 256], f32)
            nc.sync.dma_start(stat_g[:], cc_out[:])

            bng_s = cp.tile([1, D], f32)
            nc.sync.dma_start(bng_s[:], bng[:])
            bnb_s = cp.tile([1, D], f32)
            nc.sync.dma_start(bnb_s[:], bnb[:])

            mean = cp.tile([1, D], f32)
            nc.vector.tensor_scalar_mul(mean[:], stat_g[:, 0:D], 1.0 / N_NODES)
            ex2 = cp.tile([1, D], f32)
            nc.vector.tensor_scalar_mul(ex2[:], stat_g[:, D:256], 1.0 / N_NODES)
            msq = cp.tile([1, D], f32)
            nc.vector.tensor_tensor(msq[:], mean[:], mean[:], op=OP.mult)
            var = cp.tile([1, D], f32)
            nc.vector.tensor_tensor(var[:], ex2[:], msq[:], op=OP.subtract)
            vpe = cp.tile([1, D], f32)
            nc.vector.tensor_scalar_add(vpe[:], var[:], BN_EPS)
            lnv = cp.tile([1, D], f32)
            nc.scalar.activation(lnv[:], vpe[:], AF.Ln)
            rstd = cp.tile([1, D], f32)
            nc.scalar.activation(rstd[:], lnv[:], AF.Exp, scale=-0.5)
            scale_r = cp.tile([1, D], f32)
            nc.vector.tensor_tensor(scale_r[:], bng_s[:], rstd[:], op=OP.mult)
            mscl = cp.tile([1, D], f32)
            nc.vector.tensor_tensor(mscl[:], mean[:], scale_r[:], op=OP.mult)
            shift_r = cp.tile([1, D], f32)
            nc.vector.tensor_tensor(shift_r[:], bnb_s[:], mscl[:], op=OP.subtract)

            sc_t = cp.tile([P, D], f32)
            nc.gpsimd.partition_broadcast(sc_t[:], scale_r[:])
            sh_t = cp.tile([P, D], f32)
            nc.gpsimd.partition_broadcast(sh_t[:], shift_r[:])

            # residual h via on-chip transpose of hTs; softplus output
            for q in range((NW + 3) // 4):
                q0 = q * 4
                qn = min(4, NW - q0)
                hq = wp.tile([P, 4 * P], f16, tag="hq")
                nc.sync.dma_start(hq[:, :qn * P], hTs[:, q0 * P:(q0 + qn) * P])
                ps_h = ptr.tile([P, 4, D], f16, tag="tr")
                for j in range(qn):
                    nc.tensor.transpose(ps_h[:, j, :], hq[:, j * P:(j + 1) * P], ident[:])
                hres32 = wp.tile([P, 4, D], f32, tag="hres32")
                nc.scalar.copy(hres32[:, :qn, :], ps_h[:, :qn, :])
                t1 = wp.tile([P, 4, D], f32, tag="t1")
                for j in range(qn):
                    nc.vector.tensor_tensor(t1[:, j, :], agg[:, q0 + j, :], sc_t[:], op=OP.mult)
                    nc.vector.tensor_tensor(t1[:, j, :], t1[:, j, :], sh_t[:], op=OP.add)
                nc.vector.tensor_tensor(t1[:, :qn, :], t1[:, :qn, :], hres32[:, :qn, :], op=OP.add)
                t2 = wp.tile([P, 4, D], f32, tag="t2")
                nc.scalar.activation(t2[:, :qn, :], t1[:, :qn, :], AF.Exp)
                t3 = wp.tile([P, 4, D], f32, tag="t3")
                nc.scalar.activation(t3[:, :qn, :], t2[:, :qn, :], AF.Ln, bias=1.0)
                for j in range(qn):
                    nc.sync.dma_start(out_d[(q0 + j) * P:(q0 + j + 1) * P, :], t3[:, j, :])

    nc.compile()
    return nc


_NC_CACHE = None
_NC_LAYOUT = None


def kernel(h, edge_index, edge_feat, gate_w, gate_b, cand_w, cand_b,
           bn_gamma, bn_beta):
    global _NC_CACHE, _NC_LAYOUT
    from concourse.bass_utils import run_bass_kernel_spmd

    h = np.asarray(h, dtype=np.float32)
    ei = np.asarray(edge_index)
    src = ei[0].astype(np.int64)
    dst = ei[1].astype(np.int64)
    ef = np.asarray(edge_feat, dtype=np.float32)
    gw = np.asarray(gate_w, dtype=np.float32)
    gb = np.asarray(gate_b, dtype=np.float32)
    cw = np.asarray(cand_w, dtype=np.float32)
    cb = np.asarray(cand_b, dtype=np.float32)
    gam = np.asarray(bn_gamma, dtype=np.float32).reshape(1, D)
    bet = np.asarray(bn_beta, dtype=np.float32).reshape(1, D)

    wsrc = np.concatenate([-gw[0:D], cw[0:D]], axis=1).astype(np.float16)
    wdst = np.concatenate([-gw[D:2 * D], cw[D:2 * D]], axis=1).astype(np.float16)
    wef_h = np.concatenate([-gw[2 * D:], cw[2 * D:]], axis=1).astype(np.float16)
    bias = np.concatenate([-gb, cb]).astype(np.float32)[None, :].repeat(128, 0)

    h16 = h.astype(np.float16)
    ef_t = ef.T.astype(np.float16)

    LO_T, HI_T, per_core = plan_layout(src, dst)
    layout = (tuple(LO_T), tuple(HI_T))

    in_maps = []
    for k in range(NCORES):
        idx_all_k, eft_k = pack_core(per_core[k], LO_T, HI_T, ef_t)
        h16r = np.zeros((HROWS, D), dtype=np.float16)
        uniq = per_core[k][4]
        h16r[:uniq.shape[0]] = h16[uniq]
        base = k * NB
        hTs16 = np.zeros((D, SROWS), dtype=np.float16)
        hTs16[:, :NB] = h.T[:, base:base + NB].astype(np.float16)
        in_maps.append({
            "h16r": h16r, "hTs": hTs16, "wsrc": wsrc, "wdst": wdst,
            "wef": wef_h, "bias": bias, "eft": eft_k, "idx_all": idx_all_k,
            "bng": gam, "bnb": bet,
        })

    _setup_act_tables()
    if _NC_CACHE is None or _NC_LAYOUT != layout:
        _NC_CACHE = _build_nc(LO_T, HI_T)
        _NC_LAYOUT = layout
    res = run_bass_kernel_spmd(_NC_CACHE, in_maps, core_ids=list(range(NCORES)))
    out = np.concatenate([res.results[k]["out"][:NB] for k in range(NCORES)], axis=0)
    return out.astype(np.float32)
